# revision 57
# baseline (speedup 1.0000x reference)
"""Chamfer loss kernel for Trainium2 (8 NeuronCores).

Problem: preds [8, 8192, 3], gts [8, 8192, 3] (f32).
  P[b,n,m] = ||gts[b,n] - preds[b,m]||^2
  loss = sum_{b,m} min_n P[b,n,m] + sum_{b,n} min_m P[b,n,m]

Sharding: data-parallel over batch B -- one batch element per core; the host
sums the 8 per-core partial losses.

Per-core algorithm (SINGLE distance-matrix pass; both min directions are
reduced from the same PSUM tiles, halving TensorE work vs the two-pass
variant and letting ACT+DVE share the drain):
  The full squared distance is computed on TensorE as a K=18 bf16 matmul that
  is exact to ~f32 (split-fp32 encoding; xx inside the matmul keeps values
  near the min small positive, so bf16 staging is loss-free).

  For each of 64 gt-blocks (128 gts x 8192 preds, 4 PSUM slots of 2048):
    - ACT copies the PSUM slots to SBUF bf16 (stage), except the last DB
      columns: DVE tensor_scalar reads those from PSUM f32 directly (1x),
      writing the bf16 stage as its elementwise out and that slice's per-gt
      row-min as accum (rebalances ACT vs DVE so EVERY block is identical
      and both engines stay saturated).
    - DVE tensor_tensor (packed 2x) folds the staged block into a running
      column accumulator acc[128, 8192] (elementwise min across gt-blocks)
      in two halves (the first issued mid-block); gt-block 0 uses
      tensor_copy (4x) instead, avoiding an init memset.
    - One DVE tensor_scalar (packed 4x) min-reduces the rest of the staged
      block in place, accum_out -> per-gt row-min partial.
  Tail: acc is PE-transposed in [128,128] blocks (bf16 PSUM out), ACT-staged
  back to SBUF, and DVE row-min-reduced at packed 4x -> per-pred mins. The
  last gt-block's colacc is split per chunk so the tail transposes start
  while the main loop drains.

This walrus build only accepts ONE sync-wait per instruction, so the BIR
json is post-processed to hoist extra waits onto EventSemaphore carriers.
"""

import json

import numpy as np
import ml_dtypes

BF16 = ml_dtypes.bfloat16

B, N, M, D = 8, 8192, 8192, 3
P = 128           # partitions per gt-block
NT = N // P       # 64 gt-blocks
CH = 512          # one PSUM bank of f32 (matmul max moving free dim)
SLOT = 2048       # PSUM drain slot (4 banks); 2 pool slots fill all of PSUM
NSLOT = M // SLOT  # 4 slots per gt-block
K = 18            # matmul contraction rows (split-fp32 encoding)
NCORES = 8
BIG = 3.0e38
# Per block, the last DB columns skip the ACT stage copy: DVE reads the
# PSUM f32 directly (1x tensor_scalar), writing the bf16 stage as its
# elementwise out and that slice's row-min as accum. Tuned so ACT and DVE
# busy time come out equal in EVERY block (uniform blocks pipeline cleanly;
# a mix of block shapes loses the faster engine's slack to jitter).
DB = 472


def _split_bf16(x):
    hi = x.astype(BF16)
    lo = (x - hi.astype(np.float32)).astype(BF16)
    return hi, lo


def _split3_bf16(x):
    """x (f32) -> three bf16 arrays summing to ~x (residual ~2^-27 rel)."""
    hi = x.astype(BF16)
    r1 = x - hi.astype(np.float32)
    mid = r1.astype(BF16)
    r2 = r1 - mid.astype(np.float32)
    lo = r2.astype(BF16)
    return hi, mid, lo


def _build_pass(a_pts, b_pts):
    """lhsT [K,N] bf16, rhs [K,M] bf16 with
    lhsT.T @ rhs ~= ||a||^2 - 2 a.b + ||b||^2  (full squared distance)."""
    a = a_pts.astype(np.float32)
    b = b_pts.astype(np.float32)
    t = -2.0 * a
    t_hi, t_lo = _split_bf16(t)
    p_hi, p_lo = _split_bf16(b)
    yy = (b * b).sum(-1, dtype=np.float32)
    yy_hi, yy_mid, yy_lo = _split3_bf16(yy)
    xx = (a * a).sum(-1, dtype=np.float32)
    xx_hi, xx_mid, xx_lo = _split3_bf16(xx)
    ones_a = np.ones((a.shape[0],), dtype=BF16)
    ones_b = np.ones((b.shape[0],), dtype=BF16)

    lhsT = np.stack(
        [t_hi[:, 0], t_hi[:, 1], t_hi[:, 2],
         t_hi[:, 0], t_hi[:, 1], t_hi[:, 2],
         t_lo[:, 0], t_lo[:, 1], t_lo[:, 2],
         t_lo[:, 0], t_lo[:, 1], t_lo[:, 2],
         ones_a, ones_a, ones_a,
         xx_hi, xx_mid, xx_lo]
    )
    rhs = np.stack(
        [p_hi[:, 0], p_hi[:, 1], p_hi[:, 2],
         p_lo[:, 0], p_lo[:, 1], p_lo[:, 2],
         p_hi[:, 0], p_hi[:, 1], p_hi[:, 2],
         p_lo[:, 0], p_lo[:, 1], p_lo[:, 2],
         yy_hi, yy_mid, yy_lo,
         ones_b, ones_b, ones_b]
    )
    return lhsT, rhs


MAX_WAITS = 1

# Compute engines execute and complete in order, and the hardware already
# serializes same-engine data hazards (per-op DRAIN / access bubbles), so a
# compute instruction's wait on its OWN engine's completion semaphore is
# redundant -- dropping it avoids an EventSemaphore carrier on the hot path.
_COMPUTE_OPS = {"Activation", "TensorScalarPtr", "TensorReduce",
                "TensorTensor", "TensorCopy", "Matmult", "Ldweights",
                "Memset"}


def _split_waits_json(raw: bytes) -> bytes:
    """Drop redundant same-engine waits on compute ops, then hoist any
    remaining extra sync-waits onto EventSemaphore carriers (this walrus
    build rejects instructions with more than one wait)."""
    d = json.loads(raw)
    for f in d["functions"]:
        for blk in f["blocks"]:
            insts = blk.get("instructions")
            if not insts:
                continue
            new = []
            changed = False
            for inst in insts:
                si = inst.get("sync_info")
                waits = (si or {}).get("on_wait") or []
                eng = inst.get("engine", "")
                if (len(waits) > MAX_WAITS
                        and inst.get("opcode") in _COMPUTE_OPS
                        and eng not in ("SP", "Unassigned")):
                    kept = [w for w in waits
                            if not (w.get("ant_name") or "").startswith(eng + "_")]
                    if len(kept) != len(waits):
                        si["on_wait"] = waits = kept
                        changed = True
                if len(waits) > MAX_WAITS:
                    extra = waits[:-MAX_WAITS]
                    keep = waits[-MAX_WAITS:]
                    for k, w in enumerate(extra):
                        new.append({
                            "debug": inst.get("debug", 0),
                            "engine": inst["engine"],
                            "ins": [], "outs": [],
                            "name": f"{inst['name']}_sw{k}",
                            "opcode": "EventSemaphore",
                            "sync_info": {"on_wait": [w], "on_update": []},
                        })
                    si["on_wait"] = keep
                    changed = True
                new.append(inst)
            if changed:
                blk["instructions"] = new
    return json.dumps(d).encode()


def _build_nc():
    import concourse.bass as bass
    import concourse.tile as tile
    import concourse.mybir as mybir

    f32 = mybir.dt.float32
    bf16 = mybir.dt.bfloat16
    MIN = mybir.AluOpType.min
    X = mybir.AxisListType.X

    nc = bass.Bass()
    dram = {}
    for nm in ("lA", "rA"):
        dram[nm] = nc.dram_tensor(nm, [K, 8192], bf16, kind="ExternalInput")
    iden = nc.dram_tensor("iden", [P, P], bf16, kind="ExternalInput")
    out = nc.dram_tensor("out", [P, 2 * NT], f32, kind="ExternalOutput")

    with tile.TileContext(nc) as tc:
        with (
            tc.tile_pool(name="const", bufs=1) as cpool,
            tc.tile_pool(name="stage", bufs=3) as stpool,
        ):
            wt = cpool.tile([K, 8192], bf16, name="wA", tag="wA")
            rt = cpool.tile([K, 8192], bf16, name="rA", tag="rA")
            # input loads: DMA transfers serialize on the shared DMA
            # engines, so order by need -- a tiny first lhsT piece, then all
            # rhs chunks (block 0 consumes them at drain pace), then the
            # rest. All on the SP queue so the ACT sequencer stays clear
            # for the stage copies; idt is only needed at the tail.
            nc.sync.dma_start(wt[:, 0:512], dram["lA"][:, 0:512])
            for c in range(4):
                s = slice(2048 * c, 2048 * (c + 1))
                nc.sync.dma_start(rt[:, s], dram["rA"][:, s])
            nc.sync.dma_start(wt[:, 512:2048], dram["lA"][:, 512:2048])
            for c in range(1, 4):
                s = slice(2048 * c, 2048 * (c + 1))
                nc.sync.dma_start(wt[:, s], dram["lA"][:, s])
            idt = cpool.tile([P, P], bf16, name="idt", tag="idt")
            nc.sync.dma_start(idt[:], iden[:])
            acc = cpool.tile([P, M], bf16, name="acc", tag="acc")
            minv = cpool.tile([P, 2 * NT], f32, name="minv", tag="minv")
            partsD = cpool.tile([P, 2 * NT], f32, name="partsD", tag="partsD")

            with tc.tile_pool(name="psum", bufs=2, space="PSUM") as psum:
                for i in range(NT):
                    lhsT = wt[:, P * i:P * (i + 1)]
                    stg = stpool.tile([P, M], bf16, name="stg", tag="stg")
                    for c in range(NSLOT):
                        W = psum.tile([P, SLOT], f32, name="W", tag="W")
                        for h in range(SLOT // CH):
                            m0 = SLOT * c + CH * h
                            nc.tensor.matmul(
                                W[:, CH * h:CH * (h + 1)],
                                lhsT, rt[:, m0:m0 + CH],
                                start=True, stop=True,
                            )
                        sc = stg[:, SLOT * c:SLOT * (c + 1)]
                        if c == NSLOT - 1:
                            # last DB cols bypass ACT: fused stage+row-min
                            # straight from PSUM (1x)
                            nc.scalar.copy(sc[:, 0:SLOT - DB],
                                           W[:, 0:SLOT - DB])
                            nc.vector.tensor_scalar(
                                sc[:, SLOT - DB:], W[:, SLOT - DB:],
                                BIG, None, op0=MIN, op1=MIN,
                                accum_out=partsD[:, 2 * i:2 * i + 1])
                        else:
                            nc.scalar.copy(sc, W[:])
                        # column-accumulate the first staged half early so
                        # DVE starts mid-block instead of waiting for the
                        # whole stage
                        if c == 1:
                            if i == 0:
                                nc.vector.tensor_copy(
                                    acc[:, 0:2 * SLOT], stg[:, 0:2 * SLOT])
                            else:
                                nc.vector.tensor_tensor(
                                    acc[:, 0:2 * SLOT], stg[:, 0:2 * SLOT],
                                    acc[:, 0:2 * SLOT], op=MIN)
                    # in-place row-min over the ACT-staged columns (4x)
                    nc.vector.tensor_scalar(
                        stg[:, 0:M - DB], stg[:, 0:M - DB], BIG, None,
                        op0=MIN, op1=MIN,
                        accum_out=partsD[:, 2 * i + 1:2 * i + 2])
                    # the last block's colacc is split per 2048-chunk so the
                    # tail transposes of each m-range can start early
                    hs = ([(2 * SLOT, 3 * SLOT), (3 * SLOT, 4 * SLOT)]
                          if i == NT - 1 else [(2 * SLOT, 4 * SLOT)])
                    for (h0, h1) in hs:
                        if i == 0:
                            nc.vector.tensor_copy(
                                acc[:, h0:h1], stg[:, h0:h1])
                        else:
                            nc.vector.tensor_tensor(
                                acc[:, h0:h1], stg[:, h0:h1],
                                acc[:, h0:h1], op=MIN)
                # per-gt mins: merge the two row-min partials of every block
                nc.vector.tensor_reduce(
                    minv[:, 0:NT],
                    partsD[:].rearrange("p (g n) -> p g n", n=2),
                    axis=X, op=MIN,
                )
            nc.sync.dma_start(out[:, 0:NT], minv[:, 0:NT])

            # tail: per-pred mins = partition-reduce of acc via PE transpose,
            # ACT-staged back to SBUF so DVE reduces at packed 4x
            with (
                tc.tile_pool(name="psum2", bufs=4, space="PSUM") as psum2,
                tc.tile_pool(name="tsb", bufs=4) as tpool,
            ):
                TGRP = 16
                for tg in range(NT // TGRP):
                    T = psum2.tile([P, TGRP * P], bf16, name="T", tag="T")
                    for u in range(TGRP):
                        b0 = (tg * TGRP + u) * P
                        nc.tensor.transpose(
                            T[:, u * P:(u + 1) * P], acc[:, b0:b0 + P], idt[:])
                    ts = tpool.tile([P, TGRP * P], bf16, name="ts", tag="ts")
                    nc.scalar.copy(ts[:], T[:])
                    for u in range(TGRP):
                        b = tg * TGRP + u
                        nc.vector.tensor_scalar(
                            ts[:, u * P:(u + 1) * P], ts[:, u * P:(u + 1) * P],
                            BIG, None, op0=MIN, op1=MIN,
                            accum_out=minv[:, NT + b:NT + b + 1])
            nc.sync.dma_start(out[:, NT:], minv[:, NT:])

    orig = nc.to_json_bytes
    nc.to_json_bytes = lambda: _split_waits_json(orig())
    return nc


_LAST_RESULTS = None


def _prepare_in_maps(preds, gts):
    iden = np.eye(P, dtype=np.float32).astype(BF16)
    in_maps = []
    for b in range(B):
        lA, rA = _build_pass(gts[b], preds[b])
        in_maps.append({"lA": lA, "rA": rA, "iden": iden})
    return in_maps


def kernel(preds, gts, _trace=False):
    from concourse.bass_utils import run_bass_kernel_spmd

    global _LAST_RESULTS
    preds = np.asarray(preds)
    gts = np.asarray(gts)
    assert preds.shape == (B, M, D) and gts.shape == (B, N, D)

    in_maps = _prepare_in_maps(preds, gts)
    last_err = None
    for attempt in range(4):
        try:
            nc = _build_nc()
            res = run_bass_kernel_spmd(
                nc, in_maps, core_ids=list(range(NCORES)), trace=_trace,
            )
            break
        except Exception as e:         # transient device errors clear on retry
            last_err = e
            import time
            time.sleep(5 * (attempt + 1))
            try:                        # drop the wedged PJRT client state
                import jax
                jax.clear_caches()
                jax.clear_backends()
            except Exception:
                pass
    else:
        raise last_err
    _LAST_RESULTS = res

    total = 0.0
    for b in range(B):
        total += res.results[b]["out"].astype(np.float64).sum()
    return np.asarray(total, dtype=np.float32)


# ----------------------------------------------------------------------------
# Benchmark support (test-only): build the jitted sharded executable once and
# re-invoke it, so per-call wall time ~= dispatch overhead + NEFF exec time.
# ----------------------------------------------------------------------------

def _make_runner(nc, in_maps):
    import jax
    import jax.numpy as jnp
    import concourse.mybir as mybir
    from concourse import bass2jax
    from jax.experimental.shard_map import shard_map
    from jax.sharding import Mesh, PartitionSpec

    bass2jax.install_neuronx_cc_hook()
    n_cores = len(in_maps)

    partition_name = nc.partition_id_tensor.name if nc.partition_id_tensor else None
    in_names, out_names, out_avals, zero_outs = [], [], [], []
    for alloc in nc.m.functions[0].allocations:
        if not isinstance(alloc, mybir.MemoryLocationSet):
            continue
        name = alloc.memorylocations[0].name
        if alloc.kind == "ExternalInput":
            if name != partition_name:
                in_names.append(name)
        elif alloc.kind == "ExternalOutput":
            shape = tuple(alloc.tensor_shape)
            dtype = mybir.dt.np(alloc.dtype)
            out_names.append(name)
            out_avals.append(jax.core.ShapedArray(shape, dtype))
            zero_outs.append(np.zeros(shape, dtype))
    n_params = len(in_names)
    n_outs = len(out_avals)
    in_names = in_names + out_names
    if partition_name is not None:
        in_names.append(partition_name)
    donate = tuple(range(n_params, n_params + n_outs))

    def _body(*args):
        operands = list(args)
        if partition_name is not None:
            operands.append(bass2jax.partition_id_tensor())
        outs = bass2jax._bass_exec_p.bind(
            *operands,
            out_avals=tuple(out_avals),
            in_names=tuple(in_names),
            out_names=tuple(out_names),
            lowering_input_output_aliases=(),
            sim_require_finite=True,
            sim_require_nnan=True,
            nc=nc,
        )
        return tuple(outs)

    devices = jax.devices()[:n_cores]
    mesh = Mesh(np.asarray(devices), ("core",))
    in_specs = (PartitionSpec("core"),) * (n_params + n_outs)
    out_specs = (PartitionSpec("core"),) * len(out_names)
    sharded = jax.jit(
        shard_map(_body, mesh=mesh, in_specs=in_specs, out_specs=out_specs,
                  check_rep=False),
        donate_argnums=donate, keep_unused=True,
    )
    per_core = [[np.asarray(m[name]) for name in in_names[:n_params]]
                for m in in_maps]
    concat_in = [np.concatenate([per_core[c][i] for c in range(n_cores)], axis=0)
                 for i in range(n_params)]
    concat_in = jax.device_put(concat_in)
    concat_in = [jnp.asarray(a) for a in concat_in]

    def run_once():
        zeros = [np.zeros((n_cores * z.shape[0], *z.shape[1:]), z.dtype)
                 for z in zero_outs]
        outs = sharded(*concat_in, *zeros)
        jax.block_until_ready(outs)
        return [
            {name: np.asarray(outs[i]).reshape(n_cores, *out_avals[i].shape)[c]
             for i, name in enumerate(out_names)}
            for c in range(n_cores)
        ]

    return run_once


def _build_null_nc():
    """Tiny kernel used to calibrate fixed dispatch overhead."""
    import concourse.bass as bass
    import concourse.tile as tile
    import concourse.mybir as mybir

    nc = bass.Bass()
    x = nc.dram_tensor("nx", [P, 16], mybir.dt.float32, kind="ExternalInput")
    y = nc.dram_tensor("nout", [P, 16], mybir.dt.float32, kind="ExternalOutput")
    with tile.TileContext(nc) as tc:
        with tc.tile_pool(name="sb", bufs=1) as sb:
            t = sb.tile([P, 16], mybir.dt.float32, name="t", tag="t")
            nc.sync.dma_start(t[:], x[:])
            nc.sync.dma_start(y[:], t[:])
    orig = nc.to_json_bytes
    nc.to_json_bytes = lambda: _split_waits_json(orig())
    return nc


def benchmark(preds, gts, iters=30):
    """Returns (loss, per_call_times_s, null_times_s)."""
    import time

    preds = np.asarray(preds)
    gts = np.asarray(gts)
    in_maps = _prepare_in_maps(preds, gts)
    nc = _build_nc()
    run = _make_runner(nc, in_maps)

    results = run()                     # compile + first exec
    total = sum(r["out"].astype(np.float64).sum() for r in results)

    times = []
    for _ in range(iters):
        t0 = time.perf_counter()
        run()
        times.append(time.perf_counter() - t0)

    null_nc = _build_null_nc()
    null_in = [{"nx": np.zeros((P, 16), np.float32)} for _ in range(NCORES)]
    null_run = _make_runner(null_nc, null_in)
    null_run()
    null_times = []
    for _ in range(iters):
        t0 = time.perf_counter()
        null_run()
        null_times.append(time.perf_counter() - t0)

    return np.asarray(total, dtype=np.float32), times, null_times


# revision 59
# speedup vs baseline: 1.0003x; 1.0003x over previous
"""Chamfer loss kernel for Trainium2 (8 NeuronCores).

Problem: preds [8, 8192, 3], gts [8, 8192, 3] (f32).
  P[b,n,m] = ||gts[b,n] - preds[b,m]||^2
  loss = sum_{b,m} min_n P[b,n,m] + sum_{b,n} min_m P[b,n,m]

Sharding: data-parallel over batch B -- one batch element per core; the host
sums the 8 per-core partial losses.

Per-core algorithm (SINGLE distance-matrix pass; both min directions are
reduced from the same PSUM tiles, halving TensorE work vs the two-pass
variant and letting ACT+DVE share the drain):
  The full squared distance is computed on TensorE as a K=18 bf16 matmul that
  is exact to ~f32 (split-fp32 encoding; xx inside the matmul keeps values
  near the min small positive, so bf16 staging is loss-free).

  For each of 64 gt-blocks (128 gts x 8192 preds, 4 PSUM slots of 2048):
    - ACT copies the PSUM slots to SBUF bf16 (stage), except the last DB
      columns: DVE tensor_scalar reads those from PSUM f32 directly (1x),
      writing the bf16 stage as its elementwise out and that slice's per-gt
      row-min as accum (rebalances ACT vs DVE so EVERY block is identical
      and both engines stay saturated).
    - DVE tensor_tensor (packed 2x) folds the staged block into a running
      column accumulator acc[128, 8192] (elementwise min across gt-blocks)
      in two halves (the first issued mid-block); gt-block 0 uses
      tensor_copy (4x) instead, avoiding an init memset.
    - One DVE tensor_scalar (packed 4x) min-reduces the rest of the staged
      block in place, accum_out -> per-gt row-min partial.
  Tail: acc is PE-transposed in [128,128] blocks (bf16 PSUM out), ACT-staged
  back to SBUF, and DVE row-min-reduced at packed 4x -> per-pred mins. The
  last gt-block's colacc is split per chunk so the tail transposes start
  while the main loop drains.

This walrus build only accepts ONE sync-wait per instruction, so the BIR
json is post-processed to hoist extra waits onto EventSemaphore carriers.
"""

import json

import numpy as np
import ml_dtypes

BF16 = ml_dtypes.bfloat16

B, N, M, D = 8, 8192, 8192, 3
P = 128           # partitions per gt-block
NT = N // P       # 64 gt-blocks
CH = 512          # one PSUM bank of f32 (matmul max moving free dim)
SLOT = 2048       # PSUM drain slot (4 banks); 2 pool slots fill all of PSUM
NSLOT = M // SLOT  # 4 slots per gt-block
K = 18            # matmul contraction rows (split-fp32 encoding)
NCORES = 8
BIG = 3.0e38
# Per block, the last DB columns skip the ACT stage copy: DVE reads the
# PSUM f32 directly (1x tensor_scalar), writing the bf16 stage as its
# elementwise out and that slice's row-min as accum. Tuned so ACT and DVE
# busy time come out equal in EVERY block (uniform blocks pipeline cleanly;
# a mix of block shapes loses the faster engine's slack to jitter).
DB = 456


def _split_bf16(x):
    hi = x.astype(BF16)
    lo = (x - hi.astype(np.float32)).astype(BF16)
    return hi, lo


def _split3_bf16(x):
    """x (f32) -> three bf16 arrays summing to ~x (residual ~2^-27 rel)."""
    hi = x.astype(BF16)
    r1 = x - hi.astype(np.float32)
    mid = r1.astype(BF16)
    r2 = r1 - mid.astype(np.float32)
    lo = r2.astype(BF16)
    return hi, mid, lo


def _build_pass(a_pts, b_pts):
    """lhsT [K,N] bf16, rhs [K,M] bf16 with
    lhsT.T @ rhs ~= ||a||^2 - 2 a.b + ||b||^2  (full squared distance)."""
    a = a_pts.astype(np.float32)
    b = b_pts.astype(np.float32)
    t = -2.0 * a
    t_hi, t_lo = _split_bf16(t)
    p_hi, p_lo = _split_bf16(b)
    yy = (b * b).sum(-1, dtype=np.float32)
    yy_hi, yy_mid, yy_lo = _split3_bf16(yy)
    xx = (a * a).sum(-1, dtype=np.float32)
    xx_hi, xx_mid, xx_lo = _split3_bf16(xx)
    ones_a = np.ones((a.shape[0],), dtype=BF16)
    ones_b = np.ones((b.shape[0],), dtype=BF16)

    lhsT = np.stack(
        [t_hi[:, 0], t_hi[:, 1], t_hi[:, 2],
         t_hi[:, 0], t_hi[:, 1], t_hi[:, 2],
         t_lo[:, 0], t_lo[:, 1], t_lo[:, 2],
         t_lo[:, 0], t_lo[:, 1], t_lo[:, 2],
         ones_a, ones_a, ones_a,
         xx_hi, xx_mid, xx_lo]
    )
    rhs = np.stack(
        [p_hi[:, 0], p_hi[:, 1], p_hi[:, 2],
         p_lo[:, 0], p_lo[:, 1], p_lo[:, 2],
         p_hi[:, 0], p_hi[:, 1], p_hi[:, 2],
         p_lo[:, 0], p_lo[:, 1], p_lo[:, 2],
         yy_hi, yy_mid, yy_lo,
         ones_b, ones_b, ones_b]
    )
    return lhsT, rhs


MAX_WAITS = 1

# Compute engines execute and complete in order, and the hardware already
# serializes same-engine data hazards (per-op DRAIN / access bubbles), so a
# compute instruction's wait on its OWN engine's completion semaphore is
# redundant -- dropping it avoids an EventSemaphore carrier on the hot path.
_COMPUTE_OPS = {"Activation", "TensorScalarPtr", "TensorReduce",
                "TensorTensor", "TensorCopy", "Matmult", "Ldweights",
                "Memset"}


def _split_waits_json(raw: bytes) -> bytes:
    """Drop redundant same-engine waits on compute ops, then hoist any
    remaining extra sync-waits onto EventSemaphore carriers (this walrus
    build rejects instructions with more than one wait)."""
    d = json.loads(raw)
    for f in d["functions"]:
        for blk in f["blocks"]:
            insts = blk.get("instructions")
            if not insts:
                continue
            new = []
            changed = False
            for inst in insts:
                si = inst.get("sync_info")
                waits = (si or {}).get("on_wait") or []
                eng = inst.get("engine", "")
                if (len(waits) > MAX_WAITS
                        and inst.get("opcode") in _COMPUTE_OPS
                        and eng not in ("SP", "Unassigned")):
                    kept = [w for w in waits
                            if not (w.get("ant_name") or "").startswith(eng + "_")]
                    if len(kept) != len(waits):
                        si["on_wait"] = waits = kept
                        changed = True
                if len(waits) > MAX_WAITS:
                    extra = waits[:-MAX_WAITS]
                    keep = waits[-MAX_WAITS:]
                    for k, w in enumerate(extra):
                        new.append({
                            "debug": inst.get("debug", 0),
                            "engine": inst["engine"],
                            "ins": [], "outs": [],
                            "name": f"{inst['name']}_sw{k}",
                            "opcode": "EventSemaphore",
                            "sync_info": {"on_wait": [w], "on_update": []},
                        })
                    si["on_wait"] = keep
                    changed = True
                new.append(inst)
            if changed:
                blk["instructions"] = new
    return json.dumps(d).encode()


def _build_nc():
    import concourse.bass as bass
    import concourse.tile as tile
    import concourse.mybir as mybir

    f32 = mybir.dt.float32
    bf16 = mybir.dt.bfloat16
    MIN = mybir.AluOpType.min
    X = mybir.AxisListType.X

    nc = bass.Bass()
    dram = {}
    for nm in ("lA", "rA"):
        dram[nm] = nc.dram_tensor(nm, [K, 8192], bf16, kind="ExternalInput")
    iden = nc.dram_tensor("iden", [P, P], bf16, kind="ExternalInput")
    out = nc.dram_tensor("out", [P, 2 * NT], f32, kind="ExternalOutput")

    with tile.TileContext(nc) as tc:
        with (
            tc.tile_pool(name="const", bufs=1) as cpool,
            tc.tile_pool(name="stage", bufs=3) as stpool,
        ):
            wt = cpool.tile([K, 8192], bf16, name="wA", tag="wA")
            rt = cpool.tile([K, 8192], bf16, name="rA", tag="rA")
            # input loads: DMA transfers serialize on the shared DMA
            # engines, so order by need -- a tiny first lhsT piece, then all
            # rhs chunks (block 0 consumes them at drain pace), then the
            # rest. All on the SP queue so the ACT sequencer stays clear
            # for the stage copies; idt is only needed at the tail.
            nc.sync.dma_start(wt[:, 0:512], dram["lA"][:, 0:512])
            for c in range(4):
                s = slice(2048 * c, 2048 * (c + 1))
                nc.sync.dma_start(rt[:, s], dram["rA"][:, s])
            nc.sync.dma_start(wt[:, 512:2048], dram["lA"][:, 512:2048])
            for c in range(1, 4):
                s = slice(2048 * c, 2048 * (c + 1))
                nc.sync.dma_start(wt[:, s], dram["lA"][:, s])
            idt = cpool.tile([P, P], bf16, name="idt", tag="idt")
            nc.sync.dma_start(idt[:], iden[:])
            acc = cpool.tile([P, M], bf16, name="acc", tag="acc")
            minv = cpool.tile([P, 2 * NT], f32, name="minv", tag="minv")
            partsD = cpool.tile([P, 2 * NT], f32, name="partsD", tag="partsD")

            with tc.tile_pool(name="psum", bufs=2, space="PSUM") as psum:
                for i in range(NT):
                    lhsT = wt[:, P * i:P * (i + 1)]
                    stg = stpool.tile([P, M], bf16, name="stg", tag="stg")
                    for c in range(NSLOT):
                        W = psum.tile([P, SLOT], f32, name="W", tag="W")
                        for h in range(SLOT // CH):
                            m0 = SLOT * c + CH * h
                            nc.tensor.matmul(
                                W[:, CH * h:CH * (h + 1)],
                                lhsT, rt[:, m0:m0 + CH],
                                start=True, stop=True,
                            )
                        sc = stg[:, SLOT * c:SLOT * (c + 1)]
                        if c == NSLOT - 1:
                            # last DB cols bypass ACT: fused stage+row-min
                            # straight from PSUM (1x)
                            nc.scalar.copy(sc[:, 0:SLOT - DB],
                                           W[:, 0:SLOT - DB])
                            nc.vector.tensor_scalar(
                                sc[:, SLOT - DB:], W[:, SLOT - DB:],
                                BIG, None, op0=MIN, op1=MIN,
                                accum_out=partsD[:, 2 * i:2 * i + 1])
                        else:
                            nc.scalar.copy(sc, W[:])
                        # column-accumulate the first staged half early so
                        # DVE starts mid-block instead of waiting for the
                        # whole stage
                        if c == 1:
                            if i == 0:
                                nc.vector.tensor_copy(
                                    acc[:, 0:2 * SLOT], stg[:, 0:2 * SLOT])
                            else:
                                nc.vector.tensor_tensor(
                                    acc[:, 0:2 * SLOT], stg[:, 0:2 * SLOT],
                                    acc[:, 0:2 * SLOT], op=MIN)
                    # in-place row-min over the ACT-staged columns (4x)
                    nc.vector.tensor_scalar(
                        stg[:, 0:M - DB], stg[:, 0:M - DB], BIG, None,
                        op0=MIN, op1=MIN,
                        accum_out=partsD[:, 2 * i + 1:2 * i + 2])
                    # the last block's colacc is split per 2048-chunk so the
                    # tail transposes of each m-range can start early
                    hs = ([(2 * SLOT, 3 * SLOT), (3 * SLOT, 4 * SLOT)]
                          if i == NT - 1 else [(2 * SLOT, 4 * SLOT)])
                    for (h0, h1) in hs:
                        if i == 0:
                            nc.vector.tensor_copy(
                                acc[:, h0:h1], stg[:, h0:h1])
                        else:
                            nc.vector.tensor_tensor(
                                acc[:, h0:h1], stg[:, h0:h1],
                                acc[:, h0:h1], op=MIN)
                # per-gt mins: merge the two row-min partials of every block
                nc.vector.tensor_reduce(
                    minv[:, 0:NT],
                    partsD[:].rearrange("p (g n) -> p g n", n=2),
                    axis=X, op=MIN,
                )
            nc.sync.dma_start(out[:, 0:NT], minv[:, 0:NT])

            # tail: per-pred mins = partition-reduce of acc via PE transpose,
            # ACT-staged back to SBUF so DVE reduces at packed 4x
            with (
                tc.tile_pool(name="psum2", bufs=4, space="PSUM") as psum2,
                tc.tile_pool(name="tsb", bufs=4) as tpool,
            ):
                TGRP = 16
                for tg in range(NT // TGRP):
                    T = psum2.tile([P, TGRP * P], bf16, name="T", tag="T")
                    for u in range(TGRP):
                        b0 = (tg * TGRP + u) * P
                        nc.tensor.transpose(
                            T[:, u * P:(u + 1) * P], acc[:, b0:b0 + P], idt[:])
                    ts = tpool.tile([P, TGRP * P], bf16, name="ts", tag="ts")
                    nc.scalar.copy(ts[:], T[:])
                    for u in range(TGRP):
                        b = tg * TGRP + u
                        nc.vector.tensor_scalar(
                            ts[:, u * P:(u + 1) * P], ts[:, u * P:(u + 1) * P],
                            BIG, None, op0=MIN, op1=MIN,
                            accum_out=minv[:, NT + b:NT + b + 1])
            nc.sync.dma_start(out[:, NT:], minv[:, NT:])

    orig = nc.to_json_bytes
    nc.to_json_bytes = lambda: _split_waits_json(orig())
    return nc


_LAST_RESULTS = None


def _prepare_in_maps(preds, gts):
    iden = np.eye(P, dtype=np.float32).astype(BF16)
    in_maps = []
    for b in range(B):
        lA, rA = _build_pass(gts[b], preds[b])
        in_maps.append({"lA": lA, "rA": rA, "iden": iden})
    return in_maps


def kernel(preds, gts, _trace=False):
    from concourse.bass_utils import run_bass_kernel_spmd

    global _LAST_RESULTS
    preds = np.asarray(preds)
    gts = np.asarray(gts)
    assert preds.shape == (B, M, D) and gts.shape == (B, N, D)

    in_maps = _prepare_in_maps(preds, gts)
    last_err = None
    for attempt in range(4):
        try:
            nc = _build_nc()
            res = run_bass_kernel_spmd(
                nc, in_maps, core_ids=list(range(NCORES)), trace=_trace,
            )
            break
        except Exception as e:         # transient device errors clear on retry
            last_err = e
            import time
            time.sleep(5 * (attempt + 1))
            try:                        # drop the wedged PJRT client state
                import jax
                jax.clear_caches()
                jax.clear_backends()
            except Exception:
                pass
    else:
        raise last_err
    _LAST_RESULTS = res

    total = 0.0
    for b in range(B):
        total += res.results[b]["out"].astype(np.float64).sum()
    return np.asarray(total, dtype=np.float32)


# ----------------------------------------------------------------------------
# Benchmark support (test-only): build the jitted sharded executable once and
# re-invoke it, so per-call wall time ~= dispatch overhead + NEFF exec time.
# ----------------------------------------------------------------------------

def _make_runner(nc, in_maps):
    import jax
    import jax.numpy as jnp
    import concourse.mybir as mybir
    from concourse import bass2jax
    from jax.experimental.shard_map import shard_map
    from jax.sharding import Mesh, PartitionSpec

    bass2jax.install_neuronx_cc_hook()
    n_cores = len(in_maps)

    partition_name = nc.partition_id_tensor.name if nc.partition_id_tensor else None
    in_names, out_names, out_avals, zero_outs = [], [], [], []
    for alloc in nc.m.functions[0].allocations:
        if not isinstance(alloc, mybir.MemoryLocationSet):
            continue
        name = alloc.memorylocations[0].name
        if alloc.kind == "ExternalInput":
            if name != partition_name:
                in_names.append(name)
        elif alloc.kind == "ExternalOutput":
            shape = tuple(alloc.tensor_shape)
            dtype = mybir.dt.np(alloc.dtype)
            out_names.append(name)
            out_avals.append(jax.core.ShapedArray(shape, dtype))
            zero_outs.append(np.zeros(shape, dtype))
    n_params = len(in_names)
    n_outs = len(out_avals)
    in_names = in_names + out_names
    if partition_name is not None:
        in_names.append(partition_name)
    donate = tuple(range(n_params, n_params + n_outs))

    def _body(*args):
        operands = list(args)
        if partition_name is not None:
            operands.append(bass2jax.partition_id_tensor())
        outs = bass2jax._bass_exec_p.bind(
            *operands,
            out_avals=tuple(out_avals),
            in_names=tuple(in_names),
            out_names=tuple(out_names),
            lowering_input_output_aliases=(),
            sim_require_finite=True,
            sim_require_nnan=True,
            nc=nc,
        )
        return tuple(outs)

    devices = jax.devices()[:n_cores]
    mesh = Mesh(np.asarray(devices), ("core",))
    in_specs = (PartitionSpec("core"),) * (n_params + n_outs)
    out_specs = (PartitionSpec("core"),) * len(out_names)
    sharded = jax.jit(
        shard_map(_body, mesh=mesh, in_specs=in_specs, out_specs=out_specs,
                  check_rep=False),
        donate_argnums=donate, keep_unused=True,
    )
    per_core = [[np.asarray(m[name]) for name in in_names[:n_params]]
                for m in in_maps]
    concat_in = [np.concatenate([per_core[c][i] for c in range(n_cores)], axis=0)
                 for i in range(n_params)]
    concat_in = jax.device_put(concat_in)
    concat_in = [jnp.asarray(a) for a in concat_in]

    def run_once():
        zeros = [np.zeros((n_cores * z.shape[0], *z.shape[1:]), z.dtype)
                 for z in zero_outs]
        outs = sharded(*concat_in, *zeros)
        jax.block_until_ready(outs)
        return [
            {name: np.asarray(outs[i]).reshape(n_cores, *out_avals[i].shape)[c]
             for i, name in enumerate(out_names)}
            for c in range(n_cores)
        ]

    return run_once


def _build_null_nc():
    """Tiny kernel used to calibrate fixed dispatch overhead."""
    import concourse.bass as bass
    import concourse.tile as tile
    import concourse.mybir as mybir

    nc = bass.Bass()
    x = nc.dram_tensor("nx", [P, 16], mybir.dt.float32, kind="ExternalInput")
    y = nc.dram_tensor("nout", [P, 16], mybir.dt.float32, kind="ExternalOutput")
    with tile.TileContext(nc) as tc:
        with tc.tile_pool(name="sb", bufs=1) as sb:
            t = sb.tile([P, 16], mybir.dt.float32, name="t", tag="t")
            nc.sync.dma_start(t[:], x[:])
            nc.sync.dma_start(y[:], t[:])
    orig = nc.to_json_bytes
    nc.to_json_bytes = lambda: _split_waits_json(orig())
    return nc


def benchmark(preds, gts, iters=30):
    """Returns (loss, per_call_times_s, null_times_s)."""
    import time

    preds = np.asarray(preds)
    gts = np.asarray(gts)
    in_maps = _prepare_in_maps(preds, gts)
    nc = _build_nc()
    run = _make_runner(nc, in_maps)

    results = run()                     # compile + first exec
    total = sum(r["out"].astype(np.float64).sum() for r in results)

    times = []
    for _ in range(iters):
        t0 = time.perf_counter()
        run()
        times.append(time.perf_counter() - t0)

    null_nc = _build_null_nc()
    null_in = [{"nx": np.zeros((P, 16), np.float32)} for _ in range(NCORES)]
    null_run = _make_runner(null_nc, null_in)
    null_run()
    null_times = []
    for _ in range(iters):
        t0 = time.perf_counter()
        null_run()
        null_times.append(time.perf_counter() - t0)

    return np.asarray(total, dtype=np.float32), times, null_times
